# revision 1
# baseline (speedup 1.0000x reference)
"""Trainium2 Bass kernel for LuluAttention (gated GQA attention + RoPE).

Sharding over 8 NeuronCores: core = b*4 + g where b = batch (2), g = head
group (4). Each core computes 4 Q heads + their shared KV head for one batch
element, plus the matching gate slice, and a partial o_proj output
(contraction over its 512 attn dims). Host sums the 4 partials per batch.

All on-chip tensors are kept in transposed layout ([dim, seq]) so the
attention pipeline needs no on-chip transposes:
  qT/kT [d, s]  -> scoresT[sk, sq] = kT_tile.T @ qT_chunk
  softmax over sk (partition dim): denominator via ones-matmul, broadcast of
  the reciprocal via a K=1 matmul.
  v kept straight [s, d] -> attnT[d, sq] = v_tile.T @ probsT
  agT[d, sq] = attnT * gateT * recip  feeds o_proj directly as lhsT.
RoPE rotate-half needs a cross-partition rotation by 64: done with two DMA
copies, signs folded into the host-precomputed sin table.
"""

import numpy as np
import ml_dtypes
from contextlib import ExitStack

import concourse.bass as bass
import concourse.bacc as bacc
import concourse.tile as tile
from concourse import mybir
from concourse.bass_utils import run_bass_kernel_spmd

BF16 = ml_dtypes.bfloat16

HIDDEN = 2048
B = 2
S_FULL = 2048
P = 128
CH = 512               # seq chunk width
QH = 4                 # q heads per core
DQ = QH * P            # 512 q dims per core
KT = HIDDEN // P       # 16 contraction tiles
SCALE = 1.0 / float(np.sqrt(128.0))
ROPE_THETA = 10000.0


def build_program(S=S_FULL):
    f32 = mybir.dt.float32
    bf16 = mybir.dt.bfloat16
    sig = mybir.ActivationFunctionType.Sigmoid
    expf = mybir.ActivationFunctionType.Exp

    NCH = S // CH
    ST = CH // P           # 4 seq sub-tiles per chunk

    nc = bacc.Bacc("TRN2", debug=False, target_bir_lowering=False)

    xT = nc.declare_dram_parameter("xT", [HIDDEN, S], bf16, False)
    wq = nc.declare_dram_parameter("wq", [HIDDEN, DQ], bf16, False)
    wk = nc.declare_dram_parameter("wk", [HIDDEN, P], bf16, False)
    wv = nc.declare_dram_parameter("wv", [HIDDEN, P], bf16, False)
    wg = nc.declare_dram_parameter("wg", [HIDDEN, DQ], bf16, False)
    wo = nc.declare_dram_parameter("wo", [DQ, HIDDEN], bf16, False)
    bg = nc.declare_dram_parameter("bg", [DQ], f32, False)
    cosT = nc.declare_dram_parameter("cosT", [P, S], f32, False)
    sinT = nc.declare_dram_parameter("sinT", [P, S], f32, False)
    msk = nc.declare_dram_parameter("msk", [ST, P, CH], bf16, False)
    out = nc.declare_dram_parameter("out", [S, HIDDEN], f32, True)

    with tile.TileContext(nc) as tc, ExitStack() as ctx:
        wpool = ctx.enter_context(tc.tile_pool(name="weights", bufs=1))
        xpool = ctx.enter_context(tc.tile_pool(name="xchunks", bufs=2))
        qkv = ctx.enter_context(tc.tile_pool(name="qkv", bufs=1))
        work = ctx.enter_context(tc.tile_pool(name="work", bufs=3))
        agp = ctx.enter_context(tc.tile_pool(name="agp", bufs=2))
        outp = ctx.enter_context(tc.tile_pool(name="outp", bufs=2))
        ps_mm = ctx.enter_context(tc.tile_pool(name="ps_mm", bufs=2, space="PSUM"))
        ps_sc = ctx.enter_context(tc.tile_pool(name="ps_sc", bufs=2, space="PSUM"))
        ps_at = ctx.enter_context(tc.tile_pool(name="ps_at", bufs=2, space="PSUM"))
        ps_sm = ctx.enter_context(tc.tile_pool(name="ps_sm", bufs=1, space="PSUM"))

        # ---- persistent loads ----
        wq_sb = wpool.tile([P, KT, DQ], bf16, tag="wq")
        nc.sync.dma_start(out=wq_sb, in_=wq[:, :].rearrange("(kt p) n -> p kt n", p=P))
        wk_sb = wpool.tile([P, KT, P], bf16, tag="wk")
        nc.sync.dma_start(out=wk_sb, in_=wk[:, :].rearrange("(kt p) n -> p kt n", p=P))
        wv_sb = wpool.tile([P, KT, P], bf16, tag="wv")
        nc.sync.dma_start(out=wv_sb, in_=wv[:, :].rearrange("(kt p) n -> p kt n", p=P))
        wg_sb = wpool.tile([P, KT, DQ], bf16, tag="wg")
        nc.sync.dma_start(out=wg_sb, in_=wg[:, :].rearrange("(kt p) n -> p kt n", p=P))
        wo_sb = wpool.tile([P, QH, HIDDEN], bf16, tag="wo")
        nc.sync.dma_start(out=wo_sb, in_=wo[:, :].rearrange("(dt p) n -> p dt n", p=P))
        bg_sb = wpool.tile([P, QH], f32, tag="bg")
        nc.sync.dma_start(out=bg_sb, in_=bg[:].rearrange("(h p) -> p h", p=P))
        cos_sb = wpool.tile([P, S], f32, tag="cos")
        nc.sync.dma_start(out=cos_sb, in_=cosT[:, :])
        sin_sb = wpool.tile([P, S], f32, tag="sin")
        nc.sync.dma_start(out=sin_sb, in_=sinT[:, :])
        msk_sb = wpool.tile([P, ST, CH], bf16, tag="msk")
        nc.sync.dma_start(out=msk_sb, in_=msk[:, :, :].rearrange("o p n -> p o n"))
        ones_pv = wpool.tile([P, 1], bf16, tag="ones_pv")
        nc.vector.memset(ones_pv, 1.0)
        ones_bc = wpool.tile([1, P], f32, tag="ones_bc")
        nc.vector.memset(ones_bc, 1.0)

        # persistent per-core activations (transposed layouts)
        qro = qkv.tile([P, QH, S], bf16, tag="qro")
        kro = qkv.tile([P, S], bf16, tag="kro")
        v_sb = qkv.tile([P, S // P, P], bf16, tag="v")
        gt = qkv.tile([P, QH, S], bf16, tag="gt")

        for c in range(NCH):
            cs = slice(c * CH, (c + 1) * CH)

            # ---- projections for this seq chunk ----
            xc = xpool.tile([P, KT, CH], bf16, tag="xc")
            nc.sync.dma_start(
                out=xc, in_=xT[:, cs].rearrange("(kt p) n -> p kt n", p=P)
            )

            # q heads + k, with RoPE applied out of PSUM
            for qh in range(QH + 1):
                ps = ps_mm.tile([P, CH], f32, tag="proj")
                for kt in range(KT):
                    lhs = (
                        wq_sb[:, kt, qh * P:(qh + 1) * P]
                        if qh < QH
                        else wk_sb[:, kt, :]
                    )
                    nc.tensor.matmul(
                        ps, lhs, xc[:, kt, :], start=(kt == 0), stop=(kt == KT - 1)
                    )
                qf = work.tile([P, CH], f32, tag="qf")
                nc.scalar.copy(out=qf, in_=ps)
                rot = work.tile([P, CH], f32, tag="rot")
                nc.sync.dma_start(out=rot[0:64, :], in_=qf[64:128, :])
                nc.sync.dma_start(out=rot[64:128, :], in_=qf[0:64, :])
                t1 = work.tile([P, CH], f32, tag="t1")
                nc.vector.tensor_mul(t1, qf, cos_sb[:, cs])
                t2 = work.tile([P, CH], f32, tag="t2")
                nc.vector.tensor_mul(t2, rot, sin_sb[:, cs])
                dst = qro[:, qh, cs] if qh < QH else kro[:, cs]
                nc.vector.tensor_add(dst, t1, t2)

            # gate heads: sigmoid(x @ Wg + bg), transposed layout
            for qh in range(QH):
                ps = ps_mm.tile([P, CH], f32, tag="proj")
                for kt in range(KT):
                    nc.tensor.matmul(
                        ps,
                        wg_sb[:, kt, qh * P:(qh + 1) * P],
                        xc[:, kt, :],
                        start=(kt == 0),
                        stop=(kt == KT - 1),
                    )
                nc.scalar.activation(
                    out=gt[:, qh, cs],
                    in_=ps,
                    func=sig,
                    bias=bg_sb[:, qh:qh + 1],
                    scale=1.0,
                )

            # v in straight layout [s, d]
            for st in range(ST):
                s0 = c * ST + st
                ps = ps_mm.tile([P, P], f32, tag="proj")
                for kt in range(KT):
                    nc.tensor.matmul(
                        ps,
                        xc[:, kt, st * P:(st + 1) * P],
                        wv_sb[:, kt, :],
                        start=(kt == 0),
                        stop=(kt == KT - 1),
                    )
                nc.scalar.copy(out=v_sb[:, s0, :], in_=ps)

            # ---- attention for this sq chunk ----
            ag = agp.tile([P, QH, CH], bf16, tag="ag")
            ntiles = (c + 1) * ST
            for qh in range(QH):
                at = ps_at.tile([P, CH], f32, tag="attn")
                dn = ps_sm.tile([1, CH], f32, tag="denom")
                for t in range(ntiles):
                    sc_ps = ps_sc.tile([P, CH], f32, tag="sc")
                    nc.tensor.matmul(
                        sc_ps,
                        kro[:, t * P:(t + 1) * P],
                        qro[:, qh, cs],
                        start=True,
                        stop=True,
                    )
                    pr = work.tile([P, CH], bf16, tag="probs")
                    nc.scalar.activation(out=pr, in_=sc_ps, func=expf, scale=SCALE)
                    o = t - c * ST
                    if o >= 0:
                        nc.vector.tensor_mul(pr, pr, msk_sb[:, o, :])
                    nc.tensor.matmul(
                        at, v_sb[:, t, :], pr,
                        start=(t == 0), stop=(t == ntiles - 1),
                    )
                    nc.tensor.matmul(
                        dn, ones_pv, pr,
                        start=(t == 0), stop=(t == ntiles - 1),
                    )
                rc = work.tile([1, CH], f32, tag="recip")
                nc.vector.reciprocal(rc, dn)
                bc = ps_sm.tile([P, CH], f32, tag="bcast")
                nc.tensor.matmul(bc, ones_bc, rc, start=True, stop=True)
                t3 = work.tile([P, CH], f32, tag="t3")
                nc.vector.tensor_mul(t3, at, gt[:, qh, cs])
                nc.vector.tensor_mul(ag[:, qh, :], t3, bc)

            # ---- partial o_proj for this chunk ----
            for st in range(ST):
                r0 = c * CH + st * P
                for hp in range(HIDDEN // CH // 2):
                    pss = [
                        ps_mm.tile([P, CH], f32, tag="proj", name=f"ops{hi}")
                        for hi in range(2)
                    ]
                    for dt in range(QH):
                        for hi in range(2):
                            h0 = hp * 2 + hi
                            nc.tensor.matmul(
                                pss[hi],
                                ag[:, dt, st * P:(st + 1) * P],
                                wo_sb[:, dt, h0 * CH:(h0 + 1) * CH],
                                start=(dt == 0),
                                stop=(dt == QH - 1),
                            )
                    for hi in range(2):
                        h0 = hp * 2 + hi
                        ob = outp.tile([P, CH], f32, tag="ob")
                        nc.vector.tensor_copy(out=ob, in_=pss[hi])
                        nc.sync.dma_start(
                            out=out[r0:r0 + P, h0 * CH:(h0 + 1) * CH], in_=ob
                        )

    nc.finalize()
    return nc


_PROGRAMS = {}


def _get_program(S=S_FULL):
    if S not in _PROGRAMS:
        _PROGRAMS[S] = build_program(S)
    return _PROGRAMS[S]


def _host_tables(position_ids_b, S):
    pos = np.asarray(position_ids_b, dtype=np.float32)  # [S]
    inv = 1.0 / (ROPE_THETA ** (np.arange(0, P, 2, dtype=np.float32) / P))  # [64]
    ang = np.concatenate([inv, inv]).astype(np.float32)[:, None] * pos[None, :]
    cosT = np.cos(ang).astype(np.float32)
    sgn = np.where(np.arange(P) < 64, -1.0, 1.0).astype(np.float32)
    sinT = (np.sin(ang) * sgn[:, None]).astype(np.float32)
    return cosT, sinT


def _causal_masks():
    o = np.arange(CH // P)[:, None, None]
    r = np.arange(P)[None, :, None]
    j = np.arange(CH)[None, None, :]
    return ((P * o + r) <= j).astype(BF16)


def make_in_maps(x, position_ids, Wq, Wk, Wv, Wo, Wg, bg, S=S_FULL):
    x = np.asarray(x, dtype=np.float32)
    msk = _causal_masks()
    maps = []
    xT_b = [np.ascontiguousarray(x[b, :S].T).astype(BF16) for b in range(B)]
    tabs = [_host_tables(np.asarray(position_ids)[b, :S], S) for b in range(B)]
    Wq = np.asarray(Wq, np.float32)
    Wk = np.asarray(Wk, np.float32)
    Wv = np.asarray(Wv, np.float32)
    Wo = np.asarray(Wo, np.float32)
    Wg = np.asarray(Wg, np.float32)
    bg = np.asarray(bg, np.float32)
    for core in range(8):
        b, g = core // 4, core % 4
        cosT, sinT = tabs[b]
        maps.append({
            "xT": xT_b[b],
            "wq": np.ascontiguousarray(Wq[:, g * DQ:(g + 1) * DQ]).astype(BF16),
            "wk": np.ascontiguousarray(Wk[:, g * P:(g + 1) * P]).astype(BF16),
            "wv": np.ascontiguousarray(Wv[:, g * P:(g + 1) * P]).astype(BF16),
            "wg": np.ascontiguousarray(Wg[:, g * DQ:(g + 1) * DQ]).astype(BF16),
            "wo": np.ascontiguousarray(Wo[g * DQ:(g + 1) * DQ, :]).astype(BF16),
            "bg": np.ascontiguousarray(bg[g * DQ:(g + 1) * DQ]),
            "cosT": cosT,
            "sinT": sinT,
            "msk": msk,
        })
    return maps


def run(inputs, S=S_FULL, trace=False, **kw):
    nc = _get_program(S)
    maps = make_in_maps(S=S, **inputs)
    res = run_bass_kernel_spmd(nc, maps, core_ids=list(range(8)), trace=trace, **kw)
    out = np.zeros((B, S, HIDDEN), np.float32)
    for core in range(8):
        out[core // 4] += np.asarray(res.results[core]["out"], np.float32)
    return out, res


def kernel(x, position_ids, Wq, Wk, Wv, Wo, Wg, bg):
    out, _ = run(dict(x=x, position_ids=position_ids, Wq=Wq, Wk=Wk, Wv=Wv,
                      Wo=Wo, Wg=Wg, bg=bg))
    return out



# revision 5
# speedup vs baseline: 1.2154x; 1.2154x over previous
"""Trainium2 Bass kernel for LuluAttention (gated GQA attention + RoPE).

Sharding over 8 NeuronCores: core = b*4 + g where b = batch (2), g = head
group (4). Each core computes 4 Q heads + their shared KV head for one batch
element, plus the matching gate slice, and a partial o_proj output
(contraction over its 512 attn dims). Host sums the 4 partials per batch.

On-chip layouts are transposed ([dim, seq]) so the attention pipeline needs
no transposes:
  qT/kT [d, s] -> scoresT[sk, sq] = kT_tile.T @ qT_chunk
  softmax denominator via ones-matmul (partition reduction), broadcast of the
  denominator via a K=1 bf16 matmul; the reciprocal is taken on the broadcast
  [128, 512] tile (partition-parallel) and fused with the sigmoid gate:
    ag = at / ((1 + exp(-z_gate)) * denom)
  v kept straight [s, d] -> attnT[d, sq] = v_tile.T @ probsT.

Perf structure:
  - All DRAM tensors are host-pre-arranged into their exact SBUF layouts so
    every DMA is contiguous per partition (128 big descriptors per load).
  - Causal narrowing: for diagonal k-tiles only columns sq >= o*128 are
    computed in scores/exp/AV/denominator; the remaining triangular mask is a
    single [128,128] multiply.
  - Scores are issued two k-tiles ahead of the AV matmuls so the scalar
    engine's exp latency is hidden behind PE work.
  - RoPE rotate-half (cross-partition move by 64) via DVE stream_shuffle;
    signs folded into the host-precomputed sin table.
"""

import numpy as np
import ml_dtypes
from contextlib import ExitStack

import concourse.bass as bass
import concourse.bacc as bacc
import concourse.tile as tile
from concourse import mybir
from concourse.bass_utils import run_bass_kernel_spmd

BF16 = ml_dtypes.bfloat16

HIDDEN = 2048
B = 2
S_FULL = 2048
P = 128
CH = 512               # seq chunk width
QH = 4                 # q heads per core
DQ = QH * P            # 512 q dims per core
KT = HIDDEN // P       # 16 contraction tiles
SCALE = 1.0 / float(np.sqrt(128.0))
ROPE_THETA = 10000.0

IDENT32 = list(range(32))


def build_program(S=S_FULL):
    f32 = mybir.dt.float32
    bf16 = mybir.dt.bfloat16
    expf = mybir.ActivationFunctionType.Exp

    NCH = S // CH
    ST = CH // P           # 4 seq sub-tiles per chunk

    nc = bacc.Bacc("TRN2", debug=False, target_bir_lowering=False)

    xT = nc.declare_dram_parameter("xT", [NCH, P, KT, CH], bf16, False)
    wq = nc.declare_dram_parameter("wq", [P, KT, DQ], bf16, False)
    wk = nc.declare_dram_parameter("wk", [P, KT, P], bf16, False)
    wv = nc.declare_dram_parameter("wv", [P, KT, P], bf16, False)
    wg = nc.declare_dram_parameter("wg", [P, KT, DQ], bf16, False)
    wo = nc.declare_dram_parameter("wo", [P, QH, HIDDEN], bf16, False)
    bgn = nc.declare_dram_parameter("bgn", [P, QH], f32, False)
    cosT = nc.declare_dram_parameter("cosT", [P, S], f32, False)
    sinT = nc.declare_dram_parameter("sinT", [P, S], f32, False)
    msk = nc.declare_dram_parameter("msk", [P, P], bf16, False)
    out = nc.declare_dram_parameter("out", [S, HIDDEN], f32, True)

    with tile.TileContext(nc) as tc, ExitStack() as ctx:
        wpool = ctx.enter_context(tc.tile_pool(name="weights", bufs=1))
        xpool = ctx.enter_context(tc.tile_pool(name="xchunks", bufs=2))
        qkv = ctx.enter_context(tc.tile_pool(name="qkv", bufs=1))
        egp = ctx.enter_context(tc.tile_pool(name="eg", bufs=2))
        work = ctx.enter_context(tc.tile_pool(name="work", bufs=2))
        prp = ctx.enter_context(tc.tile_pool(name="probs", bufs=4))
        nrm = ctx.enter_context(tc.tile_pool(name="nrm", bufs=2))
        agp = ctx.enter_context(tc.tile_pool(name="agp", bufs=2))
        outp = ctx.enter_context(tc.tile_pool(name="outp", bufs=2))
        ps_mm = ctx.enter_context(tc.tile_pool(name="ps_mm", bufs=2, space="PSUM"))
        ps_sc = ctx.enter_context(tc.tile_pool(name="ps_sc", bufs=3, space="PSUM"))
        ps_at = ctx.enter_context(tc.tile_pool(name="ps_at", bufs=1, space="PSUM"))
        ps_dn = ctx.enter_context(tc.tile_pool(name="ps_dn", bufs=1, space="PSUM"))

        # ---- persistent loads; x chunk 0 first so k/v proj start ASAP ----
        xcs = [None] * NCH
        xcs[0] = xpool.tile([P, KT, CH], bf16, tag="xc", name="xc0")
        nc.sync.dma_start(out=xcs[0], in_=xT[0, :, :, :])
        wk_sb = wpool.tile([P, KT, P], bf16, tag="wk")
        nc.sync.dma_start(out=wk_sb, in_=wk[:, :, :])
        wv_sb = wpool.tile([P, KT, P], bf16, tag="wv")
        nc.sync.dma_start(out=wv_sb, in_=wv[:, :, :])
        cos_sb = wpool.tile([P, S], f32, tag="cos")
        nc.sync.dma_start(out=cos_sb, in_=cosT[:, :])
        sin_sb = wpool.tile([P, S], f32, tag="sin")
        nc.sync.dma_start(out=sin_sb, in_=sinT[:, :])
        wq_sb = wpool.tile([P, KT, DQ], bf16, tag="wq")
        nc.sync.dma_start(out=wq_sb, in_=wq[:, :, :])
        msk_sb = wpool.tile([P, P], bf16, tag="msk")
        nc.sync.dma_start(out=msk_sb, in_=msk[:, :])
        wg_sb = wpool.tile([P, KT, DQ], bf16, tag="wg")
        nc.sync.dma_start(out=wg_sb, in_=wg[:, :, :])
        bgn_sb = wpool.tile([P, QH], f32, tag="bgn")
        nc.sync.dma_start(out=bgn_sb, in_=bgn[:, :])
        wo_sb = wpool.tile([P, QH, HIDDEN], bf16, tag="wo")
        nc.sync.dma_start(out=wo_sb, in_=wo[:, :, :])
        ones_pv = wpool.tile([P, 1], bf16, tag="ones_pv")
        nc.vector.memset(ones_pv, 1.0)
        ones_bc = wpool.tile([1, P], bf16, tag="ones_bc")
        nc.vector.memset(ones_bc, 1.0)

        # persistent per-core activations (transposed layouts)
        qro = qkv.tile([P, QH, S], bf16, tag="qro")
        kro = qkv.tile([P, S], bf16, tag="kro")
        v_sb = qkv.tile([P, S // P, P], bf16, tag="v")

        for c in range(NCH):
            cs = slice(c * CH, (c + 1) * CH)
            xc = xcs[c]

            def rope_head(ps, dst):
                qf = work.tile([P, CH], f32, tag="qf")
                nc.scalar.copy(out=qf, in_=ps)
                rot = work.tile([P, CH], f32, tag="rot")
                nc.vector.stream_shuffle(rot[0:64, :], qf[64:128, :], IDENT32)
                nc.vector.stream_shuffle(rot[64:128, :], qf[0:64, :], IDENT32)
                t1 = work.tile([P, CH], f32, tag="t1")
                nc.vector.tensor_mul(t1, qf, cos_sb[:, cs])
                t2 = work.tile([P, CH], f32, tag="t2")
                nc.vector.tensor_mul(t2, rot, sin_sb[:, cs])
                nc.vector.tensor_add(dst, t1, t2)

            # ---- k projection + RoPE ----
            psk = ps_mm.tile([P, CH], f32, tag="proj")
            for kt in range(KT):
                nc.tensor.matmul(
                    psk, wk_sb[:, kt, :], xc[:, kt, :],
                    start=(kt == 0), stop=(kt == KT - 1),
                )
            rope_head(psk, kro[:, cs])

            # ---- v projection (straight layout [s, d]) ----
            # 4 st-tiles land in disjoint 128-col regions of one PSUM bank
            psv = ps_mm.tile([P, CH], f32, tag="proj", name="psv")
            for st in range(ST):
                for kt in range(KT):
                    nc.tensor.matmul(
                        psv[:, st * P:(st + 1) * P],
                        xc[:, kt, st * P:(st + 1) * P], wv_sb[:, kt, :],
                        start=(kt == 0), stop=(kt == KT - 1),
                    )
            nc.scalar.copy(out=v_sb[:, c * ST:(c + 1) * ST, :], in_=psv)

            # ---- q heads + RoPE ----
            for qh in range(QH):
                psq = ps_mm.tile([P, CH], f32, tag="proj")
                for kt in range(KT):
                    nc.tensor.matmul(
                        psq, wq_sb[:, kt, qh * P:(qh + 1) * P], xc[:, kt, :],
                        start=(kt == 0), stop=(kt == KT - 1),
                    )
                rope_head(psq, qro[:, qh, cs])

            # ---- gate heads: Eg = exp(-(z + bg)); sigmoid folded into norm ----
            eg = egp.tile([P, QH, CH], bf16, tag="eg")
            for qh in range(QH):
                psg = ps_mm.tile([P, CH], f32, tag="proj")
                for kt in range(KT):
                    nc.tensor.matmul(
                        psg, wg_sb[:, kt, qh * P:(qh + 1) * P], xc[:, kt, :],
                        start=(kt == 0), stop=(kt == KT - 1),
                    )
                nc.scalar.activation(
                    out=eg[:, qh, :], in_=psg, func=expf,
                    bias=bgn_sb[:, qh:qh + 1], scale=-1.0,
                )

            # prefetch next x chunk while attention runs
            if c + 1 < NCH:
                xcs[c + 1] = xpool.tile([P, KT, CH], bf16, tag="xc",
                                        name=f"xc{c + 1}")
                nc.sync.dma_start(out=xcs[c + 1], in_=xT[c + 1, :, :, :])

            # ---- attention for this sq chunk ----
            ag = agp.tile([P, QH, CH], bf16, tag="ag")
            ntiles = (c + 1) * ST
            for qh in range(QH):
                at = ps_at.tile([P, CH], f32, tag="attn")
                dn = ps_dn.tile([1, CH], f32, tag="denom")
                sc_slots = {}

                def issue_sc(t, qh=qh):
                    o = t - c * ST
                    lo = o * P if o > 0 else 0
                    s = ps_sc.tile([P, CH], f32, tag="sc")
                    nc.tensor.matmul(
                        s[:, lo:], kro[:, t * P:(t + 1) * P],
                        qro[:, qh, c * CH + lo:(c + 1) * CH],
                        start=True, stop=True,
                    )
                    sc_slots[t] = (s, lo)

                issue_sc(0)
                if ntiles > 1:
                    issue_sc(1)
                for t in range(ntiles):
                    s, lo = sc_slots.pop(t)
                    o = t - c * ST
                    pr = prp.tile([P, CH], bf16, tag="pr")
                    nc.scalar.activation(
                        out=pr[:, lo:], in_=s[:, lo:], func=expf, scale=SCALE,
                    )
                    if o >= 0:
                        nc.vector.tensor_mul(
                            pr[:, o * P:(o + 1) * P],
                            pr[:, o * P:(o + 1) * P], msk_sb,
                        )
                    if t + 2 < ntiles:
                        issue_sc(t + 2)
                    nc.tensor.matmul(
                        at[:, lo:], v_sb[:, t, :], pr[:, lo:],
                        start=(t == 0), stop=(t == ntiles - 1),
                    )
                    nc.tensor.matmul(
                        dn[:, lo:], ones_pv, pr[:, lo:],
                        start=(t == 0), stop=(t == ntiles - 1),
                    )

                # normalization fused with the sigmoid gate:
                # ag = at / ((1 + Eg) * denom)
                dnsb = nrm.tile([1, CH], bf16, tag="dnsb")
                nc.scalar.copy(out=dnsb, in_=dn)
                bc = ps_sc.tile([P, CH], f32, tag="sc", name="bc")
                nc.tensor.matmul(bc, ones_bc, dnsb, start=True, stop=True)
                w = nrm.tile([P, CH], f32, tag="w")
                nc.vector.scalar_tensor_tensor(
                    out=w, in0=eg[:, qh, :], scalar=1.0, in1=bc,
                    op0=mybir.AluOpType.add, op1=mybir.AluOpType.mult,
                )
                r = nrm.tile([P, CH], f32, tag="r")
                nc.vector.reciprocal(r, w)
                nc.vector.tensor_mul(ag[:, qh, :], at, r)

            # ---- partial o_proj for this chunk ----
            for st in range(ST):
                r0 = c * CH + st * P
                obt = outp.tile([P, HIDDEN], f32, tag="obt")
                for hp in range(2):
                    pss = [
                        ps_mm.tile([P, CH], f32, tag="proj", name=f"ops{hi}")
                        for hi in range(2)
                    ]
                    for dt in range(QH):
                        for hi in range(2):
                            h0 = hp * 2 + hi
                            nc.tensor.matmul(
                                pss[hi],
                                ag[:, dt, st * P:(st + 1) * P],
                                wo_sb[:, dt, h0 * CH:(h0 + 1) * CH],
                                start=(dt == 0), stop=(dt == QH - 1),
                            )
                    for hi in range(2):
                        h0 = hp * 2 + hi
                        if hi == 0:
                            nc.vector.tensor_copy(
                                out=obt[:, h0 * CH:(h0 + 1) * CH], in_=pss[hi]
                            )
                        else:
                            nc.scalar.copy(
                                out=obt[:, h0 * CH:(h0 + 1) * CH], in_=pss[hi]
                            )
                nc.sync.dma_start(out=out[r0:r0 + P, :], in_=obt)

    nc.finalize()
    return nc


_PROGRAMS = {}


def _get_program(S=S_FULL):
    if S not in _PROGRAMS:
        _PROGRAMS[S] = build_program(S)
    return _PROGRAMS[S]


def _host_tables(position_ids_b, S):
    pos = np.asarray(position_ids_b, dtype=np.float32)  # [S]
    inv = 1.0 / (ROPE_THETA ** (np.arange(0, P, 2, dtype=np.float32) / P))  # [64]
    ang = np.concatenate([inv, inv]).astype(np.float32)[:, None] * pos[None, :]
    cosT = np.cos(ang).astype(np.float32)
    sgn = np.where(np.arange(P) < 64, -1.0, 1.0).astype(np.float32)
    sinT = (np.sin(ang) * sgn[:, None]).astype(np.float32)
    return cosT, sinT


def make_in_maps(x, position_ids, Wq, Wk, Wv, Wo, Wg, bg, S=S_FULL):
    NCH = S // CH
    x = np.asarray(x, dtype=np.float32)
    msk = (np.arange(P)[:, None] <= np.arange(P)[None, :]).astype(BF16)
    xT_b = []
    for b in range(B):
        xt = np.ascontiguousarray(x[b, :S].T)                    # [H, S]
        xt = xt.reshape(KT, P, NCH, CH).transpose(2, 1, 0, 3)    # [NCH,P,KT,CH]
        xT_b.append(np.ascontiguousarray(xt).astype(BF16))
    tabs = [_host_tables(np.asarray(position_ids)[b, :S], S) for b in range(B)]
    Wq = np.asarray(Wq, np.float32)
    Wk = np.asarray(Wk, np.float32)
    Wv = np.asarray(Wv, np.float32)
    Wo = np.asarray(Wo, np.float32)
    Wg = np.asarray(Wg, np.float32)
    bg = np.asarray(bg, np.float32)

    def warr(w):  # [H, N] -> [P, KT, N]
        n = w.shape[1]
        return np.ascontiguousarray(
            w.reshape(KT, P, n).transpose(1, 0, 2)).astype(BF16)

    maps = []
    for core in range(8):
        b, g = core // 4, core % 4
        cosT, sinT = tabs[b]
        wo_c = Wo[g * DQ:(g + 1) * DQ, :].reshape(QH, P, HIDDEN).transpose(1, 0, 2)
        bgn_c = (-bg[g * DQ:(g + 1) * DQ]).reshape(QH, P).T
        maps.append({
            "xT": xT_b[b],
            "wq": warr(Wq[:, g * DQ:(g + 1) * DQ]),
            "wk": warr(Wk[:, g * P:(g + 1) * P]),
            "wv": warr(Wv[:, g * P:(g + 1) * P]),
            "wg": warr(Wg[:, g * DQ:(g + 1) * DQ]),
            "wo": np.ascontiguousarray(wo_c).astype(BF16),
            "bgn": np.ascontiguousarray(bgn_c),
            "cosT": cosT,
            "sinT": sinT,
            "msk": msk,
        })
    return maps


def run(inputs, S=S_FULL, trace=False, **kw):
    nc = _get_program(S)
    maps = make_in_maps(S=S, **inputs)
    res = run_bass_kernel_spmd(nc, maps, core_ids=list(range(8)), trace=trace, **kw)
    out = np.zeros((B, S, HIDDEN), np.float32)
    for core in range(8):
        out[core // 4] += np.asarray(res.results[core]["out"], np.float32)
    return out, res


def kernel(x, position_ids, Wq, Wk, Wv, Wo, Wg, bg):
    out, _ = run(dict(x=x, position_ids=position_ids, Wq=Wq, Wk=Wk, Wv=Wv,
                      Wo=Wo, Wg=Wg, bg=bg))
    return out


# revision 8
# speedup vs baseline: 1.4694x; 1.2090x over previous
"""Trainium2 Bass kernel for LuluAttention (gated GQA attention + RoPE).

Sharding over 8 NeuronCores: core = b*4 + g where b = batch (2), g = head
group (4). Each core computes 4 Q heads + their shared KV head for one batch
element, plus the matching gate slice, and a partial o_proj output
(contraction over its 512 attn dims). Host sums the 4 partials per batch.

On-chip layouts are transposed ([dim, seq]) so the attention pipeline needs
no transposes:
  qT/kT [d, s] -> scoresT[sk, sq] = kT_tile.T @ qT_chunk
  softmax denominator via ones-matmul (partition reduction), broadcast of the
  denominator via a K=1 bf16 matmul; the reciprocal is taken on the broadcast
  [128, 512] tile (partition-parallel) and fused with the sigmoid gate:
    ag = at / ((1 + exp(-z_gate)) * denom)
  v kept straight [s, d] -> attnT[d, sq] = v_tile.T @ probsT.

Perf structure:
  - All DRAM tensors are host-pre-arranged into their exact SBUF layouts so
    every DMA is contiguous per partition (128 big descriptors per load).
  - Causal narrowing: for diagonal k-tiles only columns sq >= o*128 are
    computed in scores/exp/AV/denominator; the remaining triangular mask is a
    single [128,128] multiply.
  - Scores are issued two k-tiles ahead of the AV matmuls so the scalar
    engine's exp latency is hidden behind PE work.
  - RoPE rotate-half (cross-partition move by 64) via DVE stream_shuffle;
    signs folded into the host-precomputed sin table.
"""

import numpy as np
import ml_dtypes
from contextlib import ExitStack

import concourse.bass as bass
import concourse.bacc as bacc
import concourse.tile as tile
from concourse import mybir
from concourse.bass_utils import run_bass_kernel_spmd

BF16 = ml_dtypes.bfloat16

HIDDEN = 2048
B = 2
S_FULL = 2048
P = 128
CH = 512               # seq chunk width
QH = 4                 # q heads per core
DQ = QH * P            # 512 q dims per core
KT = HIDDEN // P       # 16 contraction tiles
SCALE = 1.0 / float(np.sqrt(128.0))
ROPE_THETA = 10000.0

IDENT32 = list(range(32))


def build_program(S=S_FULL):
    f32 = mybir.dt.float32
    bf16 = mybir.dt.bfloat16
    expf = mybir.ActivationFunctionType.Exp

    NCH = S // CH
    ST = CH // P           # 4 seq sub-tiles per chunk

    nc = bacc.Bacc("TRN2", debug=False, target_bir_lowering=False)

    xT = nc.declare_dram_parameter("xT", [NCH, P, KT, CH], bf16, False)
    wq = nc.declare_dram_parameter("wq", [P, KT, DQ], bf16, False)
    wk = nc.declare_dram_parameter("wk", [P, KT, P], bf16, False)
    wv = nc.declare_dram_parameter("wv", [P, KT, P], bf16, False)
    wg = nc.declare_dram_parameter("wg", [P, KT, DQ], bf16, False)
    wo = nc.declare_dram_parameter("wo", [P, QH, HIDDEN], bf16, False)
    bgn = nc.declare_dram_parameter("bgn", [P, QH], f32, False)
    cosT = nc.declare_dram_parameter("cosT", [P, S], f32, False)
    sinT = nc.declare_dram_parameter("sinT", [P, S], f32, False)
    msk = nc.declare_dram_parameter("msk", [P, P], bf16, False)
    out = nc.declare_dram_parameter("out", [S, HIDDEN], f32, True)

    with tile.TileContext(nc) as tc, ExitStack() as ctx:
        wpool = ctx.enter_context(tc.tile_pool(name="weights", bufs=1))
        xpool = ctx.enter_context(tc.tile_pool(name="xchunks", bufs=2))
        qkv = ctx.enter_context(tc.tile_pool(name="qkv", bufs=1))
        egp = ctx.enter_context(tc.tile_pool(name="eg", bufs=2))
        work = ctx.enter_context(tc.tile_pool(name="work", bufs=2))
        prp = ctx.enter_context(tc.tile_pool(name="probs", bufs=4))
        nrm = ctx.enter_context(tc.tile_pool(name="nrm", bufs=2))
        agp = ctx.enter_context(tc.tile_pool(name="agp", bufs=2))
        outp = ctx.enter_context(tc.tile_pool(name="outp", bufs=2))
        ps_mm = ctx.enter_context(tc.tile_pool(name="ps_mm", bufs=2, space="PSUM"))
        ps_sc = ctx.enter_context(tc.tile_pool(name="ps_sc", bufs=2, space="PSUM"))
        ps_at = ctx.enter_context(tc.tile_pool(name="ps_at", bufs=2, space="PSUM"))
        ps_dn = ctx.enter_context(tc.tile_pool(name="ps_dn", bufs=1, space="PSUM"))

        # ---- persistent loads; x chunk 0 first so k/v proj start ASAP ----
        xcs = [None] * NCH
        xcs[0] = xpool.tile([P, KT, CH], bf16, tag="xc", name="xc0")
        nc.sync.dma_start(out=xcs[0], in_=xT[0, :, :, :])
        wk_sb = wpool.tile([P, KT, P], bf16, tag="wk")
        nc.sync.dma_start(out=wk_sb, in_=wk[:, :, :])
        wv_sb = wpool.tile([P, KT, P], bf16, tag="wv")
        nc.sync.dma_start(out=wv_sb, in_=wv[:, :, :])
        cos_sb = wpool.tile([P, S], f32, tag="cos")
        nc.sync.dma_start(out=cos_sb, in_=cosT[:, :])
        sin_sb = wpool.tile([P, S], f32, tag="sin")
        nc.sync.dma_start(out=sin_sb, in_=sinT[:, :])
        wq_sb = wpool.tile([P, KT, DQ], bf16, tag="wq")
        nc.sync.dma_start(out=wq_sb, in_=wq[:, :, :])
        msk_sb = wpool.tile([P, P], bf16, tag="msk")
        nc.sync.dma_start(out=msk_sb, in_=msk[:, :])
        wg_sb = wpool.tile([P, KT, DQ], bf16, tag="wg")
        nc.sync.dma_start(out=wg_sb, in_=wg[:, :, :])
        bgn_sb = wpool.tile([P, QH], f32, tag="bgn")
        nc.sync.dma_start(out=bgn_sb, in_=bgn[:, :])
        wo_sb = wpool.tile([P, QH, HIDDEN], bf16, tag="wo")
        nc.sync.dma_start(out=wo_sb, in_=wo[:, :, :])
        ones_pv = wpool.tile([P, 1], bf16, tag="ones_pv")
        nc.vector.memset(ones_pv, 1.0)
        ones_bc = wpool.tile([1, P], bf16, tag="ones_bc")
        nc.vector.memset(ones_bc, 1.0)

        # persistent per-core activations (transposed layouts)
        qro = qkv.tile([P, QH, S], bf16, tag="qro")
        kro = qkv.tile([P, S], bf16, tag="kro")
        v_sb = qkv.tile([P, S // P, P], bf16, tag="v")

        def do_oproj(ci, ag_t):
            for st in range(ST):
                r0 = ci * CH + st * P
                obt = outp.tile([P, HIDDEN], f32, tag="obt")
                for hp in range(2):
                    pss = [
                        ps_mm.tile([P, CH], f32, tag="proj", name=f"ops{hi}")
                        for hi in range(2)
                    ]
                    for dt in range(QH):
                        for hi in range(2):
                            h0 = hp * 2 + hi
                            nc.tensor.matmul(
                                pss[hi],
                                ag_t[:, dt, st * P:(st + 1) * P],
                                wo_sb[:, dt, h0 * CH:(h0 + 1) * CH],
                                start=(dt == 0), stop=(dt == QH - 1),
                            )
                    for hi in range(2):
                        h0 = hp * 2 + hi
                        if hi == 0:
                            nc.vector.tensor_copy(
                                out=obt[:, h0 * CH:(h0 + 1) * CH], in_=pss[hi]
                            )
                        else:
                            nc.scalar.copy(
                                out=obt[:, h0 * CH:(h0 + 1) * CH], in_=pss[hi]
                            )
                nc.sync.dma_start(out=out[r0:r0 + P, :], in_=obt)

        ag_prev = None

        for c in range(NCH):
            cs = slice(c * CH, (c + 1) * CH)
            xc = xcs[c]

            def rope_head(ps, dst):
                qf = work.tile([P, CH], f32, tag="qf")
                nc.scalar.copy(out=qf, in_=ps)
                rot = work.tile([P, CH], f32, tag="rot")
                nc.vector.stream_shuffle(rot[0:64, :], qf[64:128, :], IDENT32)
                nc.vector.stream_shuffle(rot[64:128, :], qf[0:64, :], IDENT32)
                t1 = work.tile([P, CH], f32, tag="t1")
                nc.vector.tensor_mul(t1, qf, cos_sb[:, cs])
                t2 = work.tile([P, CH], f32, tag="t2")
                nc.vector.tensor_mul(t2, rot, sin_sb[:, cs])
                nc.vector.tensor_add(dst, t1, t2)

            # ---- k projection + RoPE ----
            psk = ps_mm.tile([P, CH], f32, tag="proj")
            for kt in range(KT):
                nc.tensor.matmul(
                    psk, wk_sb[:, kt, :], xc[:, kt, :],
                    start=(kt == 0), stop=(kt == KT - 1),
                )
            rope_head(psk, kro[:, cs])

            # ---- v projection (straight layout [s, d]) ----
            # 4 st-tiles land in disjoint 128-col regions of one PSUM bank
            psv = ps_mm.tile([P, CH], f32, tag="proj", name="psv")
            for st in range(ST):
                for kt in range(KT):
                    nc.tensor.matmul(
                        psv[:, st * P:(st + 1) * P],
                        xc[:, kt, st * P:(st + 1) * P], wv_sb[:, kt, :],
                        start=(kt == 0), stop=(kt == KT - 1),
                    )
            nc.scalar.copy(out=v_sb[:, c * ST:(c + 1) * ST, :], in_=psv)

            # ---- q heads + RoPE ----
            for qh in range(QH):
                psq = ps_mm.tile([P, CH], f32, tag="proj")
                for kt in range(KT):
                    nc.tensor.matmul(
                        psq, wq_sb[:, kt, qh * P:(qh + 1) * P], xc[:, kt, :],
                        start=(kt == 0), stop=(kt == KT - 1),
                    )
                rope_head(psq, qro[:, qh, cs])

            # ---- gate heads: Eg = exp(-(z + bg)); sigmoid folded into norm ----
            eg = egp.tile([P, QH, CH], bf16, tag="eg")
            for qh in range(QH):
                psg = ps_mm.tile([P, CH], f32, tag="proj")
                for kt in range(KT):
                    nc.tensor.matmul(
                        psg, wg_sb[:, kt, qh * P:(qh + 1) * P], xc[:, kt, :],
                        start=(kt == 0), stop=(kt == KT - 1),
                    )
                nc.scalar.activation(
                    out=eg[:, qh, :], in_=psg, func=expf,
                    bias=bgn_sb[:, qh:qh + 1], scale=-1.0,
                )

            # prefetch next x chunk while attention runs
            if c + 1 < NCH:
                xcs[c + 1] = xpool.tile([P, KT, CH], bf16, tag="xc",
                                        name=f"xc{c + 1}")
                nc.sync.dma_start(out=xcs[c + 1], in_=xT[c + 1, :, :, :])

            # ---- deferred o_proj of the PREVIOUS chunk (its ag is long
            # ready; keeps the norm chain off the PE critical path) ----
            if c > 0:
                do_oproj(c - 1, ag_prev)

            # ---- attention for this sq chunk ----
            ag = agp.tile([P, QH, CH], bf16, tag="ag")
            ntiles = (c + 1) * ST

            # normalization fused with the sigmoid gate:
            # ag = at / ((1 + Eg) * denom).  Issued AFTER the next head's
            # score prologue so the chain overlaps the next head's t-loop.
            def norm_head(qh, at, dn):
                dnsb = nrm.tile([1, CH], bf16, tag="dnsb")
                nc.scalar.copy(out=dnsb, in_=dn)
                bc = ps_mm.tile([P, CH], f32, tag="proj", name="bc")
                nc.tensor.matmul(bc, ones_bc, dnsb, start=True, stop=True)
                w = nrm.tile([P, CH], f32, tag="w")
                nc.vector.scalar_tensor_tensor(
                    out=w, in0=eg[:, qh, :], scalar=1.0, in1=bc,
                    op0=mybir.AluOpType.add, op1=mybir.AluOpType.mult,
                )
                r = nrm.tile([P, CH], f32, tag="r")
                nc.vector.reciprocal_approx_fast(out=r, in_=w)
                nc.vector.tensor_mul(ag[:, qh, :], at, r)

            pending_norm = None
            for qh in range(QH):
                at = ps_at.tile([P, CH], f32, tag="attn")
                dn = ps_dn.tile([1, CH], f32, tag="denom")
                sc_slots = {}

                def issue_sc(t, qh=qh):
                    o = t - c * ST
                    lo = o * P if o > 0 else 0
                    s = ps_sc.tile([P, CH], f32, tag="sc")
                    nc.tensor.matmul(
                        s[:, lo:], kro[:, t * P:(t + 1) * P],
                        qro[:, qh, c * CH + lo:(c + 1) * CH],
                        start=True, stop=True,
                    )
                    sc_slots[t] = (s, lo)

                issue_sc(0)
                if ntiles > 1:
                    issue_sc(1)
                if pending_norm is not None:
                    norm_head(*pending_norm)
                for t in range(ntiles):
                    s, lo = sc_slots.pop(t)
                    o = t - c * ST
                    pr = prp.tile([P, CH], bf16, tag="pr")
                    nc.scalar.activation(
                        out=pr[:, lo:], in_=s[:, lo:], func=expf, scale=SCALE,
                    )
                    if o >= 0:
                        nc.vector.tensor_mul(
                            pr[:, o * P:(o + 1) * P],
                            pr[:, o * P:(o + 1) * P], msk_sb,
                        )
                    nc.tensor.matmul(
                        at[:, lo:], v_sb[:, t, :], pr[:, lo:],
                        start=(t == 0), stop=(t == ntiles - 1),
                    )
                    nc.tensor.matmul(
                        dn[:, lo:], ones_pv, pr[:, lo:],
                        start=(t == 0), stop=(t == ntiles - 1),
                    )
                    if t + 2 < ntiles:
                        issue_sc(t + 2)
                pending_norm = (qh, at, dn)
            norm_head(*pending_norm)
            ag_prev = ag

        # o_proj of the final chunk
        do_oproj(NCH - 1, ag_prev)

    nc.finalize()
    return nc


_PROGRAMS = {}


def _get_program(S=S_FULL):
    if S not in _PROGRAMS:
        _PROGRAMS[S] = build_program(S)
    return _PROGRAMS[S]


def _host_tables(position_ids_b, S):
    pos = np.asarray(position_ids_b, dtype=np.float32)  # [S]
    inv = 1.0 / (ROPE_THETA ** (np.arange(0, P, 2, dtype=np.float32) / P))  # [64]
    ang = np.concatenate([inv, inv]).astype(np.float32)[:, None] * pos[None, :]
    cosT = np.cos(ang).astype(np.float32)
    sgn = np.where(np.arange(P) < 64, -1.0, 1.0).astype(np.float32)
    sinT = (np.sin(ang) * sgn[:, None]).astype(np.float32)
    return cosT, sinT


def make_in_maps(x, position_ids, Wq, Wk, Wv, Wo, Wg, bg, S=S_FULL):
    NCH = S // CH
    x = np.asarray(x, dtype=np.float32)
    msk = (np.arange(P)[:, None] <= np.arange(P)[None, :]).astype(BF16)
    xT_b = []
    for b in range(B):
        xt = np.ascontiguousarray(x[b, :S].T)                    # [H, S]
        xt = xt.reshape(KT, P, NCH, CH).transpose(2, 1, 0, 3)    # [NCH,P,KT,CH]
        xT_b.append(np.ascontiguousarray(xt).astype(BF16))
    tabs = [_host_tables(np.asarray(position_ids)[b, :S], S) for b in range(B)]
    Wq = np.asarray(Wq, np.float32)
    Wk = np.asarray(Wk, np.float32)
    Wv = np.asarray(Wv, np.float32)
    Wo = np.asarray(Wo, np.float32)
    Wg = np.asarray(Wg, np.float32)
    bg = np.asarray(bg, np.float32)

    def warr(w):  # [H, N] -> [P, KT, N]
        n = w.shape[1]
        return np.ascontiguousarray(
            w.reshape(KT, P, n).transpose(1, 0, 2)).astype(BF16)

    maps = []
    for core in range(8):
        b, g = core // 4, core % 4
        cosT, sinT = tabs[b]
        wo_c = Wo[g * DQ:(g + 1) * DQ, :].reshape(QH, P, HIDDEN).transpose(1, 0, 2)
        bgn_c = (-bg[g * DQ:(g + 1) * DQ]).reshape(QH, P).T
        maps.append({
            "xT": xT_b[b],
            "wq": warr(Wq[:, g * DQ:(g + 1) * DQ]),
            "wk": warr(Wk[:, g * P:(g + 1) * P]),
            "wv": warr(Wv[:, g * P:(g + 1) * P]),
            "wg": warr(Wg[:, g * DQ:(g + 1) * DQ]),
            "wo": np.ascontiguousarray(wo_c).astype(BF16),
            "bgn": np.ascontiguousarray(bgn_c),
            "cosT": cosT,
            "sinT": sinT,
            "msk": msk,
        })
    return maps


def run(inputs, S=S_FULL, trace=False, **kw):
    nc = _get_program(S)
    maps = make_in_maps(S=S, **inputs)
    res = run_bass_kernel_spmd(nc, maps, core_ids=list(range(8)), trace=trace, **kw)
    out = np.zeros((B, S, HIDDEN), np.float32)
    for core in range(8):
        out[core // 4] += np.asarray(res.results[core]["out"], np.float32)
    return out, res


def kernel(x, position_ids, Wq, Wk, Wv, Wo, Wg, bg):
    out, _ = run(dict(x=x, position_ids=position_ids, Wq=Wq, Wk=Wk, Wv=Wv,
                      Wo=Wo, Wg=Wg, bg=bg))
    return out


# revision 13
# speedup vs baseline: 1.5135x; 1.0300x over previous
"""Trainium2 Bass kernel for LuluAttention (gated GQA attention + RoPE).

Sharding over 8 NeuronCores: core = b*4 + g where b = batch (2), g = head
group (4). Each core computes 4 Q heads + their shared KV head for one batch
element, plus the matching gate slice, and a partial o_proj output
(contraction over its 512 attn dims). Host sums the 4 partials per batch.

On-chip layouts are transposed ([dim, seq]) so the attention pipeline needs
no transposes:
  qT/kT [d, s] -> scoresT[sk, sq] = kT_tile.T @ qT_chunk
  softmax denominator via ones-matmul (partition reduction), broadcast of the
  denominator via a K=1 bf16 matmul; the reciprocal is taken on the broadcast
  [128, 512] tile (partition-parallel) and fused with the sigmoid gate:
    ag = at / ((1 + exp(-z_gate)) * denom)
  v kept straight [s, d] -> attnT[d, sq] = v_tile.T @ probsT.

Perf structure:
  - All DRAM tensors are host-pre-arranged into their exact SBUF layouts so
    every DMA is contiguous per partition (128 big descriptors per load).
  - Causal narrowing: for diagonal k-tiles only columns sq >= o*128 are
    computed in scores/exp/AV/denominator; the remaining triangular mask is a
    single [128,128] multiply.
  - Scores are issued two k-tiles ahead of the AV matmuls so the scalar
    engine's exp latency is hidden behind PE work.
  - RoPE rotate-half (cross-partition move by 64) via DVE stream_shuffle;
    signs folded into the host-precomputed sin table.
"""

import numpy as np
import ml_dtypes
from contextlib import ExitStack

import concourse.bass as bass
import concourse.bacc as bacc
import concourse.tile as tile
from concourse import mybir
from concourse.bass_utils import run_bass_kernel_spmd

BF16 = ml_dtypes.bfloat16

HIDDEN = 2048
B = 2
S_FULL = 2048
P = 128
CH = 512               # seq chunk width
QH = 4                 # q heads per core
DQ = QH * P            # 512 q dims per core
KT = HIDDEN // P       # 16 contraction tiles
SCALE = 1.0 / float(np.sqrt(128.0))
ROPE_THETA = 10000.0

IDENT32 = list(range(32))


def build_program(S=S_FULL):
    f32 = mybir.dt.float32
    bf16 = mybir.dt.bfloat16
    expf = mybir.ActivationFunctionType.Exp

    NCH = S // CH
    ST = CH // P           # 4 seq sub-tiles per chunk

    nc = bacc.Bacc("TRN2", debug=False, target_bir_lowering=False)

    xT = nc.declare_dram_parameter("xT", [NCH, P, KT, CH], bf16, False)
    wq = nc.declare_dram_parameter("wq", [P, KT, DQ], bf16, False)
    wk = nc.declare_dram_parameter("wk", [P, KT, P], bf16, False)
    wv = nc.declare_dram_parameter("wv", [P, KT, P], bf16, False)
    wg = nc.declare_dram_parameter("wg", [P, KT, DQ], bf16, False)
    wo = nc.declare_dram_parameter("wo", [P, QH, HIDDEN], bf16, False)
    bgn = nc.declare_dram_parameter("bgn", [P, QH], f32, False)
    cosT = nc.declare_dram_parameter("cosT", [P, S], f32, False)
    sinT = nc.declare_dram_parameter("sinT", [P, S], f32, False)
    msk = nc.declare_dram_parameter("msk", [P, P], bf16, False)
    out = nc.declare_dram_parameter("out", [S, HIDDEN], f32, True)

    with tile.TileContext(nc) as tc, ExitStack() as ctx:
        wpool = ctx.enter_context(tc.tile_pool(name="weights", bufs=1))
        xpool = ctx.enter_context(tc.tile_pool(name="xchunks", bufs=2))
        qkv = ctx.enter_context(tc.tile_pool(name="qkv", bufs=1))
        egp = ctx.enter_context(tc.tile_pool(name="eg", bufs=2))
        work = ctx.enter_context(tc.tile_pool(name="work", bufs=2))
        prp = ctx.enter_context(tc.tile_pool(name="probs", bufs=4))
        nrm = ctx.enter_context(tc.tile_pool(name="nrm", bufs=2))
        agp = ctx.enter_context(tc.tile_pool(name="agp", bufs=2))
        outp = ctx.enter_context(tc.tile_pool(name="outp", bufs=2))
        ps_mm = ctx.enter_context(tc.tile_pool(name="ps_mm", bufs=2, space="PSUM"))
        ps_sc = ctx.enter_context(tc.tile_pool(name="ps_sc", bufs=2, space="PSUM"))
        ps_at = ctx.enter_context(tc.tile_pool(name="ps_at", bufs=2, space="PSUM"))
        ps_dn = ctx.enter_context(tc.tile_pool(name="ps_dn", bufs=1, space="PSUM"))

        # ---- persistent loads; x chunk 0 first so k/v proj start ASAP.
        # Early loads are split into multiple dma_starts so they spread
        # across DMA queues and the first matmul isn't issue-bound. ----
        xcs = [None] * NCH
        xcs[0] = xpool.tile([P, KT, CH], bf16, tag="xc", name="xc0")
        for k0 in range(0, KT, 4):
            nc.sync.dma_start(out=xcs[0][:, k0:k0 + 4, :],
                              in_=xT[0, :, k0:k0 + 4, :])
        wk_sb = wpool.tile([P, KT, P], bf16, tag="wk")
        for k0 in range(0, KT, 8):
            nc.sync.dma_start(out=wk_sb[:, k0:k0 + 8, :],
                              in_=wk[:, k0:k0 + 8, :])
        wv_sb = wpool.tile([P, KT, P], bf16, tag="wv")
        nc.sync.dma_start(out=wv_sb, in_=wv[:, :, :])
        cos_sb = wpool.tile([P, S], f32, tag="cos")
        nc.sync.dma_start(out=cos_sb, in_=cosT[:, :])
        sin_sb = wpool.tile([P, S], f32, tag="sin")
        nc.sync.dma_start(out=sin_sb, in_=sinT[:, :])
        wq_sb = wpool.tile([P, KT, DQ], bf16, tag="wq")
        for k0 in range(0, KT, 8):
            nc.sync.dma_start(out=wq_sb[:, k0:k0 + 8, :],
                              in_=wq[:, k0:k0 + 8, :])
        msk_sb = wpool.tile([P, P], bf16, tag="msk")
        nc.sync.dma_start(out=msk_sb, in_=msk[:, :])
        wg_sb = wpool.tile([P, KT, DQ], bf16, tag="wg")
        nc.sync.dma_start(out=wg_sb, in_=wg[:, :, :])
        bgn_sb = wpool.tile([P, QH], f32, tag="bgn")
        nc.sync.dma_start(out=bgn_sb, in_=bgn[:, :])
        wo_sb = wpool.tile([P, QH, HIDDEN], bf16, tag="wo")
        nc.sync.dma_start(out=wo_sb, in_=wo[:, :, :])
        ones_pv = wpool.tile([P, 1], bf16, tag="ones_pv")
        nc.vector.memset(ones_pv, 1.0)
        ones_bc = wpool.tile([1, P], bf16, tag="ones_bc")
        nc.vector.memset(ones_bc, 1.0)

        # persistent per-core activations (transposed layouts)
        qro = qkv.tile([P, QH, S], bf16, tag="qro")
        kro = qkv.tile([P, S], bf16, tag="kro")
        v_sb = qkv.tile([P, S // P, P], bf16, tag="v")

        def do_oproj(ci, ag_t):
            for st in range(ST):
                r0 = ci * CH + st * P
                obt = outp.tile([P, HIDDEN], f32, tag="obt")
                for hp in range(2):
                    pss = [
                        ps_mm.tile([P, CH], f32, tag="proj", name=f"ops{hi}")
                        for hi in range(2)
                    ]
                    for dt in range(QH):
                        for hi in range(2):
                            h0 = hp * 2 + hi
                            nc.tensor.matmul(
                                pss[hi],
                                ag_t[:, dt, st * P:(st + 1) * P],
                                wo_sb[:, dt, h0 * CH:(h0 + 1) * CH],
                                start=(dt == 0), stop=(dt == QH - 1),
                            )
                    for hi in range(2):
                        h0 = hp * 2 + hi
                        if hi == 0:
                            nc.vector.tensor_copy(
                                out=obt[:, h0 * CH:(h0 + 1) * CH], in_=pss[hi]
                            )
                        else:
                            nc.scalar.copy(
                                out=obt[:, h0 * CH:(h0 + 1) * CH], in_=pss[hi]
                            )
                nc.sync.dma_start(out=out[r0:r0 + P, :], in_=obt)

        # normalization fused with the sigmoid gate:
        # ag = at / ((1 + Eg) * denom), split so the bc matmul never waits
        # on the dnsb scalar copy at a head boundary.
        def norm_stage2(qh, at, dnsb, eg_t, ag_t):
            bc = ps_mm.tile([P, CH], f32, tag="proj", name="bc")
            nc.tensor.matmul(bc, ones_bc, dnsb, start=True, stop=True)
            w = nrm.tile([P, CH], f32, tag="w")
            nc.vector.scalar_tensor_tensor(
                out=w, in0=eg_t[:, qh, :], scalar=1.0, in1=bc,
                op0=mybir.AluOpType.add, op1=mybir.AluOpType.mult,
            )
            r = nrm.tile([P, CH], f32, tag="r")
            nc.vector.reciprocal_approx_fast(out=r, in_=w)
            nc.vector.tensor_mul(ag_t[:, qh, :], at, r)

        ag_prev = None
        pending_norm = None

        for c in range(NCH):
            cs = slice(c * CH, (c + 1) * CH)
            xc = xcs[c]

            def rope_head(ps, dst):
                qf = work.tile([P, CH], f32, tag="qf")
                nc.scalar.copy(out=qf, in_=ps)
                rot = work.tile([P, CH], f32, tag="rot")
                nc.vector.stream_shuffle(rot[0:64, :], qf[64:128, :], IDENT32)
                nc.vector.stream_shuffle(rot[64:128, :], qf[0:64, :], IDENT32)
                t1 = work.tile([P, CH], f32, tag="t1")
                nc.vector.tensor_mul(t1, qf, cos_sb[:, cs])
                t2 = work.tile([P, CH], f32, tag="t2")
                nc.vector.tensor_mul(t2, rot, sin_sb[:, cs])
                nc.vector.tensor_add(dst, t1, t2)

            # ---- k projection + RoPE ----
            psk = ps_mm.tile([P, CH], f32, tag="proj")
            for kt in range(KT):
                nc.tensor.matmul(
                    psk, wk_sb[:, kt, :], xc[:, kt, :],
                    start=(kt == 0), stop=(kt == KT - 1),
                )
            rope_head(psk, kro[:, cs])

            # ---- deferred norm tail of the previous chunk's last head
            # (its dnsb copy is long done; the remaining proj PE work hides
            # the bc/stt/recip/mul chain completely) ----
            if pending_norm is not None:
                norm_stage2(*pending_norm)
                pending_norm = None

            # ---- v projection (straight layout [s, d]) ----
            # 4 st-tiles land in disjoint 128-col regions of one PSUM bank
            psv = ps_mm.tile([P, CH], f32, tag="proj", name="psv")
            for st in range(ST):
                for kt in range(KT):
                    nc.tensor.matmul(
                        psv[:, st * P:(st + 1) * P],
                        xc[:, kt, st * P:(st + 1) * P], wv_sb[:, kt, :],
                        start=(kt == 0), stop=(kt == KT - 1),
                    )
            nc.scalar.copy(out=v_sb[:, c * ST:(c + 1) * ST, :], in_=psv)

            # ---- q heads + RoPE ----
            for qh in range(QH):
                psq = ps_mm.tile([P, CH], f32, tag="proj")
                for kt in range(KT):
                    nc.tensor.matmul(
                        psq, wq_sb[:, kt, qh * P:(qh + 1) * P], xc[:, kt, :],
                        start=(kt == 0), stop=(kt == KT - 1),
                    )
                rope_head(psq, qro[:, qh, cs])

            # ---- gate heads: Eg = exp(-(z + bg)); sigmoid folded into norm ----
            eg = egp.tile([P, QH, CH], bf16, tag="eg")
            for qh in range(QH):
                psg = ps_mm.tile([P, CH], f32, tag="proj")
                for kt in range(KT):
                    nc.tensor.matmul(
                        psg, wg_sb[:, kt, qh * P:(qh + 1) * P], xc[:, kt, :],
                        start=(kt == 0), stop=(kt == KT - 1),
                    )
                nc.scalar.activation(
                    out=eg[:, qh, :], in_=psg, func=expf,
                    bias=bgn_sb[:, qh:qh + 1], scale=-1.0,
                )

            # prefetch next x chunk while attention runs
            if c + 1 < NCH:
                xcs[c + 1] = xpool.tile([P, KT, CH], bf16, tag="xc",
                                        name=f"xc{c + 1}")
                nc.sync.dma_start(out=xcs[c + 1], in_=xT[c + 1, :, :, :])

            # ---- deferred o_proj of the PREVIOUS chunk (its ag is long
            # ready; keeps the norm chain off the PE critical path) ----
            if c > 0:
                do_oproj(c - 1, ag_prev)

            # ---- attention for this sq chunk ----
            ag = agp.tile([P, QH, CH], bf16, tag="ag")
            ntiles = (c + 1) * ST

            for qh in range(QH):
                at = ps_at.tile([P, CH], f32, tag="attn")
                dn = ps_dn.tile([1, CH], f32, tag="denom")
                sc_slots = {}

                def issue_sc(t, qh=qh):
                    o = t - c * ST
                    lo = o * P if o > 0 else 0
                    s = ps_sc.tile([P, CH], f32, tag="sc")
                    nc.tensor.matmul(
                        s[:, lo:], kro[:, t * P:(t + 1) * P],
                        qro[:, qh, c * CH + lo:(c + 1) * CH],
                        start=True, stop=True,
                    )
                    sc_slots[t] = (s, lo)

                issue_sc(0)
                if ntiles > 1:
                    issue_sc(1)
                for t in range(ntiles):
                    s, lo = sc_slots.pop(t)
                    o = t - c * ST
                    pr = prp.tile([P, CH], bf16, tag="pr")
                    nc.scalar.activation(
                        out=pr[:, lo:], in_=s[:, lo:], func=expf, scale=SCALE,
                    )
                    if o >= 0:
                        nc.vector.tensor_mul(
                            pr[:, o * P:(o + 1) * P],
                            pr[:, o * P:(o + 1) * P], msk_sb,
                        )
                    nc.tensor.matmul(
                        at[:, lo:], v_sb[:, t, :], pr[:, lo:],
                        start=(t == 0), stop=(t == ntiles - 1),
                    )
                    nc.tensor.matmul(
                        dn[:, lo:], ones_pv, pr[:, lo:],
                        start=(t == 0), stop=(t == ntiles - 1),
                    )
                    if t + 2 < ntiles:
                        issue_sc(t + 2)
                    if t == 1 and pending_norm is not None:
                        norm_stage2(*pending_norm)
                        pending_norm = None
                # stage 1 immediately: frees the single dn bank early and
                # decouples the bc matmul from the scalar-copy latency
                dnsb = nrm.tile([1, CH], bf16, tag="dnsb")
                nc.scalar.copy(out=dnsb, in_=dn)
                pending_norm = (qh, at, dnsb, eg, ag)
            ag_prev = ag

        # final chunk tail: last head's norm, then o_proj
        norm_stage2(*pending_norm)
        do_oproj(NCH - 1, ag_prev)

    nc.finalize()
    return nc


_PROGRAMS = {}


def _get_program(S=S_FULL):
    if S not in _PROGRAMS:
        _PROGRAMS[S] = build_program(S)
    return _PROGRAMS[S]


def _host_tables(position_ids_b, S):
    pos = np.asarray(position_ids_b, dtype=np.float32)  # [S]
    inv = 1.0 / (ROPE_THETA ** (np.arange(0, P, 2, dtype=np.float32) / P))  # [64]
    ang = np.concatenate([inv, inv]).astype(np.float32)[:, None] * pos[None, :]
    cosT = np.cos(ang).astype(np.float32)
    sgn = np.where(np.arange(P) < 64, -1.0, 1.0).astype(np.float32)
    sinT = (np.sin(ang) * sgn[:, None]).astype(np.float32)
    return cosT, sinT


def make_in_maps(x, position_ids, Wq, Wk, Wv, Wo, Wg, bg, S=S_FULL):
    NCH = S // CH
    x = np.asarray(x, dtype=np.float32)
    msk = (np.arange(P)[:, None] <= np.arange(P)[None, :]).astype(BF16)
    xT_b = []
    for b in range(B):
        xt = np.ascontiguousarray(x[b, :S].T)                    # [H, S]
        xt = xt.reshape(KT, P, NCH, CH).transpose(2, 1, 0, 3)    # [NCH,P,KT,CH]
        xT_b.append(np.ascontiguousarray(xt).astype(BF16))
    tabs = [_host_tables(np.asarray(position_ids)[b, :S], S) for b in range(B)]
    Wq = np.asarray(Wq, np.float32)
    Wk = np.asarray(Wk, np.float32)
    Wv = np.asarray(Wv, np.float32)
    Wo = np.asarray(Wo, np.float32)
    Wg = np.asarray(Wg, np.float32)
    bg = np.asarray(bg, np.float32)

    def warr(w):  # [H, N] -> [P, KT, N]
        n = w.shape[1]
        return np.ascontiguousarray(
            w.reshape(KT, P, n).transpose(1, 0, 2)).astype(BF16)

    maps = []
    for core in range(8):
        b, g = core // 4, core % 4
        cosT, sinT = tabs[b]
        wo_c = Wo[g * DQ:(g + 1) * DQ, :].reshape(QH, P, HIDDEN).transpose(1, 0, 2)
        bgn_c = (-bg[g * DQ:(g + 1) * DQ]).reshape(QH, P).T
        maps.append({
            "xT": xT_b[b],
            "wq": warr(Wq[:, g * DQ:(g + 1) * DQ]),
            "wk": warr(Wk[:, g * P:(g + 1) * P]),
            "wv": warr(Wv[:, g * P:(g + 1) * P]),
            "wg": warr(Wg[:, g * DQ:(g + 1) * DQ]),
            "wo": np.ascontiguousarray(wo_c).astype(BF16),
            "bgn": np.ascontiguousarray(bgn_c),
            "cosT": cosT,
            "sinT": sinT,
            "msk": msk,
        })
    return maps


def run(inputs, S=S_FULL, trace=False, **kw):
    nc = _get_program(S)
    maps = make_in_maps(S=S, **inputs)
    res = run_bass_kernel_spmd(nc, maps, core_ids=list(range(8)), trace=trace, **kw)
    out = np.zeros((B, S, HIDDEN), np.float32)
    for core in range(8):
        out[core // 4] += np.asarray(res.results[core]["out"], np.float32)
    return out, res


def kernel(x, position_ids, Wq, Wk, Wv, Wo, Wg, bg):
    out, _ = run(dict(x=x, position_ids=position_ids, Wq=Wq, Wk=Wk, Wv=Wv,
                      Wo=Wo, Wg=Wg, bg=bg))
    return out
